# revision 1
# baseline (speedup 1.0000x reference)
"""Trainium2 Bass kernel for nn_DyHGCN (3-relation 2-layer GCN + LSTM + GCN head).

v2 architecture (8 NeuronCores, SPMD), gather-bound redesign:
  - All aggregations run as: feature-major table in SBUF [128 feats, nodes]
    -> gpsimd ap_gather (0.37 ns/idx, 15x faster than dma_gather) producing
    [128, slots] edge streams -> bucketed segmented tensor_reduce on DVE
    (segments = per-target in-edge lists padded to width%4, degree-sorted).
  - GCN algebra reordered as agg-then-W: agg_r uses tables pre-scaled by
    dinv_r[s] (host-side for x; on-device column scale for h1), the dinv[t]
    post-scale is applied as a PE-outer-product broadcast multiply. This
    kills the dense per-node W matmuls of v1 and all M matrices.
  - SPMD uniformity: per-(phase) segment-width histograms are maxed over
    cores (quota); shorter cores pad with dummy all-PADCOL segments, so one
    program fits all cores; only the index data differs.
  - Aggregation output columns are degree-sorted; later consumers absorb the
    permutation into their own index data; the final output is unpermuted on
    the host.
  - LSTM: 125 chunks x 20 steps per core, 32 warmup steps from zero state
    (host-validated rel err 5.5e-7), input taken directly from the
    feature-major osum tile (no DRAM round trip / transposes).
"""

import numpy as np

N_NODES = 20000
N_CORES = 8
F = 128
FOUT = 64
HALO = 64
LLEN = 20
LW = 32
CHUNK = 3072      # max gather slots per ap_gather call
NT0 = N_NODES     # l0 / fin table width (node-order)


# ---------------------------------------------------------------------------
# host-side preprocessing
# ---------------------------------------------------------------------------

def _csr_by_target(src, tgt, n):
    order = np.argsort(tgt, kind="stable")
    src_sorted = src[order]
    counts = np.bincount(tgt, minlength=n)
    starts = np.zeros(n + 1, np.int64)
    np.cumsum(counts, out=starts[1:])
    return src_sorted, starts


def _build_stream(widths_by_core, chunk_max=CHUNK, col_mult=None):
    """Uniform (SPMD) stream structure from per-core segment widths.

    Returns dict with:
      stream_w [nseg] widths (uniform), chunks: list of
        (slot_off, n_slots, col0, runs[(w, cnt)]), segtot, totslots,
      pos_by_width: {w: array of segment indices}.
    """
    # quota per width
    from collections import Counter
    quota = Counter()
    for wc in widths_by_core:
        c = Counter(wc.tolist())
        for w, n in c.items():
            quota[w] = max(quota[w], n)
    width_list = []
    for w in sorted(quota, reverse=True):
        width_list += [w] * quota[w]
    # chunk building (widths desc); pad each chunk tail to %16 with w4 dummies
    stream_w = []
    chunks = []
    cur_w = []
    cur_n = 0

    def close():
        nonlocal cur_w, cur_n
        pad = (16 - cur_n % 16) % 16
        while pad:
            cur_w.append(4)
            cur_n += 4
            pad -= 4
        runs = []
        for w in cur_w:
            if runs and runs[-1][0] == w:
                runs[-1][1] += 1
            else:
                runs.append([w, 1])
        chunks.append((len(stream_w), cur_n, [tuple(x) for x in runs]))
        stream_w.extend(cur_w)
        cur_w, cur_n = [], 0

    for w in width_list:
        if cur_n + w > chunk_max:
            close()
        cur_w.append(w)
        cur_n += w
    if cur_n:
        close()

    segtot = len(stream_w)
    if col_mult:
        m = (col_mult - segtot % col_mult) % col_mult
        if m:
            # m width-4 dummies; widen the last one so slot count is %16
            cur_w = [4] * m
            cur_n = 4 * m
            tail = (16 - cur_n % 16) % 16
            cur_w[-1] += tail
            cur_n += tail
            close()
            segtot = len(stream_w)
            assert segtot % col_mult == 0
    stream_w = np.asarray(stream_w, np.int64)
    seg_off = np.zeros(segtot + 1, np.int64)
    np.cumsum(stream_w, out=seg_off[1:])
    pos_by_width = {}
    for w in np.unique(stream_w):
        pos_by_width[int(w)] = np.nonzero(stream_w == w)[0]
    # final chunk descriptors with slot offsets and col0
    out_chunks = []
    for (seg0, n, runs) in chunks:
        out_chunks.append(dict(seg0=seg0, slot_off=int(seg_off[seg0]),
                               n=n, col0=seg0, runs=runs))
    return dict(stream_w=stream_w, seg_off=seg_off, chunks=out_chunks,
                segtot=segtot, totslots=int(seg_off[-1]),
                pos_by_width=pos_by_width)


def _assign_core(struct, widths_c, order_hint=None):
    """Assign this core's segments (widths_c) to stream positions.
    Returns seg_idx [ntgt]: stream segment index per target."""
    ntgt = len(widths_c)
    seg_idx = np.zeros(ntgt, np.int64)
    used = {w: 0 for w in struct["pos_by_width"]}
    # group targets by width, preserve target order within a width
    for w in np.unique(widths_c):
        tsel = np.nonzero(widths_c == w)[0]
        pos = struct["pos_by_width"][int(w)]
        k = used[int(w)]
        seg_idx[tsel] = pos[k:k + len(tsel)]
        used[int(w)] += len(tsel)
    return seg_idx


def _fill_idx(struct, seg_idx, tgt_nodes, deg, csr, colmap, padcol,
              self_col):
    """Build the int16 gather-index stream for one core/phase.

    tgt_nodes: node id per target (or -1 for dummy targets with deg 0).
    deg: in-degree (no self) per target. csr: (src_sorted, starts).
    colmap: maps node id -> table column. self_col: per-target self column
    (or -1 to skip self edge).
    """
    idx = np.full(struct["totslots"], padcol, np.int16)
    seg_off = struct["seg_off"]
    real = tgt_nodes >= 0
    rt = tgt_nodes[real]
    rdeg = deg[real]
    ss, st = csr
    starts = st[rt]
    total_e = int(rdeg.sum())
    if total_e:
        reps = np.repeat(np.arange(len(rt)), rdeg)
        intra = np.arange(total_e) - np.repeat(
            np.concatenate([[0], np.cumsum(rdeg)[:-1]]), rdeg)
        flat = starts[reps] + intra
        e_src = ss[flat]
        o = seg_off[seg_idx[real]]
        pos = np.repeat(o, rdeg) + intra
        idx[pos] = colmap[e_src].astype(np.int16)
    sc = self_col[real] if self_col is not None else None
    if sc is not None:
        idx[seg_off[seg_idx[real]] + rdeg] = sc.astype(np.int16)
    return idx


def _wrap_idx16(idx):
    n = len(idx)
    assert n % 16 == 0
    w = np.ascontiguousarray(idx.reshape(n // 16, 16).T)
    return np.tile(w, (8, 1))


def preprocess(inputs):
    SH = N_NODES // N_CORES
    x = np.asarray(inputs["x"], np.float32)
    srcs, tgts = [], []
    for r in range(3):
        ei = np.asarray(inputs[f"ei{r}"]).astype(np.int64)
        srcs.append(ei[0])
        tgts.append(ei[1])
    dinvs, csrs, degs = [], [], []
    for r in range(3):
        dg = np.bincount(tgts[r], minlength=N_NODES).astype(np.int64)
        degs.append(dg)
        dinvs.append((1.0 / np.sqrt(dg + 1.0)).astype(np.float32))
        csrs.append(_csr_by_target(srcs[r], tgts[r], N_NODES))
    all_src = np.concatenate(srcs)
    all_tgt = np.concatenate(tgts)
    deg_f = np.bincount(all_tgt, minlength=N_NODES).astype(np.int64)
    dinv_f = (1.0 / np.sqrt(deg_f + 1.0)).astype(np.float32)
    csr_f = _csr_by_target(all_src, all_tgt, N_NODES)

    def widthof(d):  # d = in-deg incl self
        return np.maximum(4, 4 * ((d + 3) // 4))

    own = [np.arange(c * SH, (c + 1) * SH, dtype=np.int64)
           for c in range(N_CORES)]
    l1tgt = []
    for c in range(N_CORES):
        halo = (np.arange(c * SH - HALO, c * SH, dtype=np.int64)
                if c > 0 else np.full(HALO, -1, np.int64))
        l1tgt.append(np.concatenate([halo, own[c]]))

    structs = {}
    for r in range(3):
        structs[("l0", r)] = _build_stream(
            [widthof(degs[r][own[c]] + 1) for c in range(N_CORES)])
    S0 = max(structs[("l0", r)]["segtot"] for r in range(3))
    # re-pad l0 streams to common segtot S0 (pad cols only matter for table
    # column addressing; easiest: use per-relation segtot, table stride = S0)
    for r in range(3):
        structs[("l1", r)] = _build_stream(
            [widthof(np.where(t >= 0, degs[r][np.maximum(t, 0)], 0) + 1)
             for c, t in enumerate(l1tgt)])
    S1 = max(structs[("l1", r)]["segtot"] for r in range(3))
    structs[("fin",)] = _build_stream(
        [widthof(deg_f[own[c]] + 1) for c in range(N_CORES)], col_mult=128)
    SF = structs[("fin",)]["segtot"]

    PAD0 = NT0            # l0/fin table pad column
    PAD1 = 8 * S0         # l1 table pad column
    NTAB1 = 8 * S0 + 1
    assert NTAB1 <= 32768 and S1 + 1 <= 32768 and SF <= 32767

    meta = dict(SH=SH, S0=S0, S1=S1, SF=SF, NTAB1=NTAB1,
                halo=HALO, llen=LLEN, lw=LW, n_cores=N_CORES,
                structs=structs)

    # per-core, per-relation l0 seg assignment (needed globally for l1 cols)
    segidx_l0 = {}   # (r, c) -> seg per own target
    colmap_l1 = {}   # r -> [N] table column in h1_r table
    for r in range(3):
        cm = np.zeros(N_NODES, np.int64)
        for c in range(N_CORES):
            w = widthof(degs[r][own[c]] + 1)
            si = _assign_core(structs[("l0", r)], w)
            segidx_l0[(r, c)] = si
            cm[own[c]] = c * S0 + si
        colmap_l1[r] = cm

    ident_colmap = np.arange(N_NODES, dtype=np.int64)

    shared = {
        "WihT": None, "WhhT": None,  # filled below
    }
    WihT = np.asarray(inputs["lstm_Wih"], np.float32)
    WhhT = np.asarray(inputs["lstm_Whh"], np.float32)
    perm = np.concatenate([np.arange(0, 128), np.arange(128, 256),
                           np.arange(384, 512), np.arange(256, 384)])
    lstm_b = (np.asarray(inputs["lstm_bih"], np.float32)
              + np.asarray(inputs["lstm_bhh"], np.float32))[perm]
    meta["lstm_bias_nonzero"] = bool(np.any(lstm_b != 0.0))
    shared = {
        "WihT": np.ascontiguousarray(WihT[perm].T),
        "WhhT": np.ascontiguousarray(WhhT[perm].T),
        "lstm_b": np.ascontiguousarray(lstm_b.reshape(4, 128).T),
        "Wo": np.asarray(inputs["Wo"], np.float32),
        "bo": np.asarray(inputs["bo"], np.float32).reshape(FOUT, 1),
        "ones1": np.ones((1, 128), np.float32),
        "ident": np.eye(128, dtype=np.float32),
    }
    for r in range(3):
        for l in range(2):
            shared[f"W_r{r}_l{l}"] = np.asarray(
                inputs[f"W_r{r}_l{l}"], np.float32)
            shared[f"b_r{r}_l{l}"] = np.asarray(
                inputs[f"b_r{r}_l{l}"], np.float32).reshape(128, 1)
        # pre-scaled transposed x tables (zero pad col handled on device)
        shared[f"xt{r}"] = np.ascontiguousarray(
            (x * dinvs[r][:, None]).T)

    in_maps = []
    fin_cols = []    # per core: column of each own target in fin stream
    for c in range(N_CORES):
        m = dict(shared)
        for r in range(3):
            st0 = structs[("l0", r)]
            si0 = segidx_l0[(r, c)]
            m[f"l0r{r}_idx"] = _wrap_idx16(_fill_idx(
                st0, si0, own[c], degs[r][own[c]], csrs[r],
                ident_colmap, PAD0, own[c]))
            dv = np.zeros(st0["segtot"], np.float32)
            dv[si0] = dinvs[r][own[c]]
            m[f"l0r{r}_dv"] = dv.reshape(1, -1)

            st1 = structs[("l1", r)]
            t1 = l1tgt[c]
            d1 = np.where(t1 >= 0, degs[r][np.maximum(t1, 0)], 0)
            w1 = widthof(np.where(t1 >= 0, d1 + 1, 0))
            si1 = _assign_core(st1, w1)
            selfc = np.where(t1 >= 0, colmap_l1[r][np.maximum(t1, 0)], 0)
            m[f"l1r{r}_idx"] = _wrap_idx16(_fill_idx(
                st1, si1, t1, d1, csrs[r], colmap_l1[r], PAD1, selfc))
            dv1 = np.zeros(st1["segtot"], np.float32)
            rl = t1 >= 0
            dv1[si1[rl]] = dinvs[r][t1[rl]]
            m[f"l1r{r}_dv"] = dv1.reshape(1, -1)
            # osum reorder: node-order j -> l1 column (core0 halo -> zerocol)
            reo = np.full(2576, S1, np.int16)
            cols = si1.astype(np.int64)
            if c == 0:
                reo[:HALO] = S1
                reo[HALO:HALO + SH] = cols[HALO:]
            else:
                reo[:HALO + SH] = cols
            m[f"reo{r}"] = _wrap_idx16(reo)

        stf = structs[("fin",)]
        wf = widthof(deg_f[own[c]] + 1)
        sif = _assign_core(stf, wf)
        fin_cols.append(sif)
        m["fin_idx"] = _wrap_idx16(_fill_idx(
            stf, sif, own[c], deg_f[own[c]], csr_f,
            ident_colmap, PAD0, own[c]))
        dvf = np.zeros(stf["segtot"], np.float32)
        dvf[sif] = dinv_f[own[c]]
        m["fin_dv"] = dvf.reshape(1, -1)
        m["dvf_own"] = dinv_f[own[c]].reshape(1, -1).astype(np.float32)
        in_maps.append(m)

    meta["fin_cols"] = fin_cols
    return in_maps, meta


# ---------------------------------------------------------------------------
# bass program
# ---------------------------------------------------------------------------

def build_program(meta, sim1=False, fake_cc=False):
    import concourse.bass as bass
    import concourse.tile as tile
    from concourse import bacc, mybir, library_config

    f32 = mybir.dt.float32
    i16 = mybir.dt.int16
    AF = mybir.ActivationFunctionType
    ALU = mybir.AluOpType

    SH = meta["SH"]
    S0, S1, SF = meta["S0"], meta["S1"], meta["SF"]
    NTAB1 = meta["NTAB1"]
    NTABMAX = max(NT0 + 1, NTAB1)
    halo, llen, lw = meta["halo"], meta["llen"], meta["lw"]
    NCH = SH // llen
    n_cores = meta["n_cores"]
    structs = meta["structs"]
    lstm_bias = meta["lstm_bias_nonzero"]
    SMAX = max(S0, S1 + 1, SF)
    rg = [list(range(n_cores))]

    nc = bacc.Bacc("TRN2", target_bir_lowering=False, debug=False,
                   num_devices=1 if sim1 else n_cores, num_swdge_queues=4)

    def inp(name, shape, dtype=f32):
        return nc.dram_tensor(name, list(shape), dtype, kind="ExternalInput")

    xt_d = [inp(f"xt{r}", (128, NT0)) for r in range(3)]
    Wz_d = {(r, l): inp(f"W_r{r}_l{l}", (128, 128))
            for r in range(3) for l in range(2)}
    bz_d = {(r, l): inp(f"b_r{r}_l{l}", (128, 1))
            for r in range(3) for l in range(2)}
    WihT_d = inp("WihT", (128, 512))
    WhhT_d = inp("WhhT", (128, 512))
    lstmb_d = inp("lstm_b", (128, 4))
    Wo_d = inp("Wo", (128, FOUT))
    bo_d = inp("bo", (FOUT, 1))
    ones1_d = inp("ones1", (1, 128))
    ident_d = inp("ident", (128, 128))
    idx_d, dv_d, reo_d = {}, {}, {}
    for r in range(3):
        idx_d[("l0", r)] = inp(f"l0r{r}_idx",
                               (128, structs[("l0", r)]["totslots"] // 16), i16)
        dv_d[("l0", r)] = inp(f"l0r{r}_dv", (1, structs[("l0", r)]["segtot"]))
        idx_d[("l1", r)] = inp(f"l1r{r}_idx",
                               (128, structs[("l1", r)]["totslots"] // 16), i16)
        dv_d[("l1", r)] = inp(f"l1r{r}_dv", (1, structs[("l1", r)]["segtot"]))
        reo_d[r] = inp(f"reo{r}", (128, 2576 // 16), i16)
    idx_d[("fin",)] = inp("fin_idx", (128, structs[("fin",)]["totslots"] // 16), i16)
    dv_d[("fin",)] = inp("fin_dv", (1, SF))
    dvfo_d = inp("dvf_own", (1, SH))

    out_d = nc.dram_tensor("out", [SF, FOUT], f32, kind="ExternalOutput")

    h1st_d = [nc.dram_tensor(f"h1st{r}", [128, S0], f32) for r in range(3)]
    h1full_d = [nc.dram_tensor(f"h1full{r}", [n_cores * 128, S0], f32,
                               addr_space="Shared") for r in range(3)]
    yst_d = nc.dram_tensor("yst", [128, SH], f32)
    yfull_d = nc.dram_tensor("yfull", [n_cores * 128, SH], f32,
                             addr_space="Shared")

    with tile.TileContext(nc) as tc:
        nc.gpsimd.load_library(library_config.ap_gather)
        import contextlib
        st = contextlib.ExitStack()
        with st:
            const = st.enter_context(tc.tile_pool(name="const", bufs=1))

            def load_const(dram, shape, dtype=f32):
                t = const.tile(list(shape), dtype, tag=dram.name,
                               name=f"c_{dram.name}")
                nc.sync.dma_start(t[:], dram.ap())
                return t

            Wz_sb = {k: load_const(v, (128, 128)) for k, v in Wz_d.items()}
            bz_sb = {k: load_const(v, (128, 1)) for k, v in bz_d.items()}
            ident_sb = load_const(ident_d, (128, 128))
            ones1_sb = load_const(ones1_d, (1, 128))
            Wo_sb = load_const(Wo_d, (128, FOUT))
            bo_sb = load_const(bo_d, (FOUT, 1))

            tab = const.tile([128, NTABMAX], f32, tag="tab")
            aggT = const.tile([128, SMAX], f32, tag="aggT")
            B1 = const.tile([128, SMAX], f32, tag="B1")
            stage = const.tile([128, SMAX], f32, tag="stage")
            osum = const.tile([128, 2576], f32, tag="osum")
            hsT = const.tile([128, SH], f32, tag="hsT")

            # ---------------- helpers --------------------------------------
            def run_agg(ph_key, ntab, gp, ixp):
                """ap_gather + segmented reduces into aggT[:, :segtot]."""
                stt = structs[ph_key]
                idxd = idx_d[ph_key]
                for ci, ch in enumerate(stt["chunks"]):
                    n = ch["n"]
                    o16 = ch["slot_off"] // 16
                    ix = ixp.tile([128, CHUNK // 16], i16, tag=f"ix{ci % 2}",
                                  name=f"ix_{ph_key}_{ci}")
                    nc.sync.dma_start(ix[:, :n // 16],
                                      idxd.ap()[:, o16:o16 + n // 16])
                    g = gp.tile([128, CHUNK], f32, tag=f"g{ci % 2}",
                                name=f"g_{ph_key}_{ci}")
                    nc.gpsimd.ap_gather(
                        g[:, :n], tab[:, :ntab], ix[:, :n // 16],
                        channels=128, num_elems=ntab, d=1, num_idxs=n)
                    col = ch["col0"]
                    off = 0
                    for (w, cnt) in ch["runs"]:
                        nc.vector.tensor_reduce(
                            aggT[:, col:col + cnt],
                            g[:, off:off + cnt * w].rearrange(
                                "p (s k) -> p s k", k=w),
                            mybir.AxisListType.X, ALU.add)
                        col += cnt
                        off += cnt * w

            def build_bcast(dst, row_d, width, pp, nparts=128):
                """dst[:nparts, :width] = ones^T @ row (broadcast row)."""
                row = const.tile([1, SMAX], f32, tag="dvrow")
                nc.sync.dma_start(row[:, :width], row_d.ap())
                for t0 in range(0, width, 512):
                    t1 = min(t0 + 512, width)
                    ps = pp.tile([128, 512], f32, tag="bc",
                                 name=f"bc{id(row_d)}_{t0}")
                    nc.tensor.matmul(ps[:nparts, :t1 - t0],
                                     ones1_sb[:, :nparts],
                                     row[:, t0:t1], start=True, stop=True)
                    nc.vector.tensor_copy(dst[:nparts, t0:t1],
                                          ps[:nparts, :t1 - t0])

            # ---------------- l0 + l1 over relations ----------------------
            with tc.tile_pool(name="gp", bufs=1) as gp, \
                 tc.tile_pool(name="ixp", bufs=1) as ixp, \
                 tc.tile_pool(name="pp", bufs=2, space="PSUM") as pp:

                nc.vector.memset(osum[:], 0.0)

                for r in range(3):
                    # table: pre-scaled x (node order) + zero pad col
                    nc.sync.dma_start(tab[:, :NT0], xt_d[r].ap())
                    nc.vector.memset(tab[:, NT0:NT0 + 1], 0.0)
                    run_agg(("l0", r), NT0 + 1, gp, ixp)
                    s0r = structs[("l0", r)]["segtot"]
                    build_bcast(B1, dv_d[("l0", r)], s0r, pp)
                    # h1 = relu((W^T agg) * dinv_t + b) * dinv_t
                    for t0 in range(0, s0r, 512):
                        t1 = min(t0 + 512, s0r)
                        ps = pp.tile([128, 512], f32, tag="mm",
                                     name=f"l0mm{r}_{t0}")
                        nc.tensor.matmul(ps[:, :t1 - t0], Wz_sb[(r, 0)][:],
                                         aggT[:, t0:t1], start=True, stop=True)
                        nc.vector.tensor_mul(ps[:, :t1 - t0], ps[:, :t1 - t0],
                                             B1[:, t0:t1])
                        nc.scalar.activation(stage[:, t0:t1], ps[:, :t1 - t0],
                                             AF.Relu, bias=bz_sb[(r, 0)][:])
                        nc.vector.tensor_mul(stage[:, t0:t1], stage[:, t0:t1],
                                             B1[:, t0:t1])
                    if s0r < S0:
                        nc.vector.memset(stage[:, s0r:S0], 0.0)
                    nc.sync.dma_start(h1st_d[r].ap(), stage[:, :S0])
                    if sim1 or fake_cc:
                        nc.sync.dma_start(h1full_d[r].ap()[0:128, :],
                                          h1st_d[r].ap())
                    else:
                        nc.gpsimd.collective_compute(
                            "AllGather", mybir.AluOpType.bypass,
                            ins=[h1st_d[r].ap()], outs=[h1full_d[r].ap()],
                            replica_groups=rg)

                for r in range(3):
                    # l1 table: AllGathered h1 (8 shards side by side) + pad
                    nc.sync.dma_start(
                        tab[:, :8 * S0].rearrange("p (c t) -> p c t", c=8),
                        h1full_d[r].ap().rearrange("(c p) t -> p c t", p=128))
                    nc.vector.memset(tab[:, 8 * S0:8 * S0 + 1], 0.0)
                    run_agg(("l1", r), NTAB1, gp, ixp)
                    s1r = structs[("l1", r)]["segtot"]
                    build_bcast(B1, dv_d[("l1", r)], s1r, pp)
                    for t0 in range(0, s1r, 512):
                        t1 = min(t0 + 512, s1r)
                        ps = pp.tile([128, 512], f32, tag="mm",
                                     name=f"l1mm{r}_{t0}")
                        nc.tensor.matmul(ps[:, :t1 - t0], Wz_sb[(r, 1)][:],
                                         aggT[:, t0:t1], start=True, stop=True)
                        nc.vector.tensor_mul(ps[:, :t1 - t0], ps[:, :t1 - t0],
                                             B1[:, t0:t1])
                        nc.scalar.activation(stage[:, t0:t1], ps[:, :t1 - t0],
                                             AF.Relu, bias=bz_sb[(r, 1)][:])
                    nc.vector.memset(stage[:, S1:S1 + 1], 0.0)
                    # reorder columns to node order and accumulate into osum
                    reo_sb = ixp.tile([128, 2576 // 16], i16, tag="reo",
                                      name=f"reo{r}")
                    nc.sync.dma_start(reo_sb[:], reo_d[r].ap())
                    g = gp.tile([128, CHUNK], f32, tag="g0", name=f"reog{r}")
                    nc.gpsimd.ap_gather(
                        g[:, :2576], stage[:, :S1 + 1], reo_sb[:],
                        channels=128, num_elems=S1 + 1, d=1, num_idxs=2576)
                    nc.vector.tensor_add(osum[:], osum[:], g[:, :2576])

                # ---------------- LSTM --------------------------------------
                with tc.tile_pool(name="pE_ps", bufs=2, space="PSUM") as ppe, \
                     tc.tile_pool(name="pE_w", bufs=1) as wp, \
                     tc.tile_pool(name="pE_s", bufs=3) as sp, \
                     tc.tile_pool(name="pE_st", bufs=3) as stp:
                    WihT_sb = wp.tile([128, 512], f32, tag="wih")
                    nc.sync.dma_start(WihT_sb[:], WihT_d.ap())
                    WhhT_sb = wp.tile([128, 512], f32, tag="whh")
                    nc.sync.dma_start(WhhT_sb[:], WhhT_d.ap())
                    if lstm_bias:
                        lb_sb = wp.tile([128, 4], f32, tag="lb")
                        nc.sync.dma_start(lb_sb[:], lstmb_d.ap())
                    h_cur = stp.tile([128, NCH], f32, tag="h")
                    c_cur = stp.tile([128, NCH], f32, tag="c")
                    nc.vector.memset(h_cur[:], 0.0)
                    nc.vector.memset(c_cur[:], 0.0)
                    for t in range(lw + llen):
                        ps = ppe.tile([128, 4 * NCH], f32, tag="gates")
                        xsl = osum[:, (halo - lw) + t:
                                   (halo - lw) + t + llen * (NCH - 1) + 1:llen]
                        for gi in range(4):
                            nc.tensor.matmul(ps[:, gi * NCH:(gi + 1) * NCH],
                                             WihT_sb[:, gi * 128:(gi + 1) * 128],
                                             xsl, start=True, stop=False)
                            nc.tensor.matmul(ps[:, gi * NCH:(gi + 1) * NCH],
                                             WhhT_sb[:, gi * 128:(gi + 1) * 128],
                                             h_cur[:], start=False, stop=True)
                        sig = sp.tile([128, 3 * NCH], f32, tag="sig")
                        gg = sp.tile([128, NCH], f32, tag="gg")
                        if lstm_bias:
                            for k in range(3):
                                nc.scalar.activation(
                                    sig[:, k * NCH:(k + 1) * NCH],
                                    ps[:, k * NCH:(k + 1) * NCH],
                                    AF.Sigmoid, bias=lb_sb[:, k:k + 1])
                            nc.scalar.activation(gg[:], ps[:, 3 * NCH:4 * NCH],
                                                 AF.Tanh, bias=lb_sb[:, 3:4])
                        else:
                            nc.scalar.activation(sig[:], ps[:, 0:3 * NCH],
                                                 AF.Sigmoid)
                            nc.scalar.activation(gg[:], ps[:, 3 * NCH:4 * NCH],
                                                 AF.Tanh)
                        fc = sp.tile([128, NCH], f32, tag="fc")
                        nc.vector.tensor_mul(fc[:], sig[:, NCH:2 * NCH], c_cur[:])
                        ig = sp.tile([128, NCH], f32, tag="ig")
                        nc.vector.tensor_mul(ig[:], sig[:, 0:NCH], gg[:])
                        c_new = stp.tile([128, NCH], f32, tag="c")
                        nc.vector.tensor_add(c_new[:], fc[:], ig[:])
                        tc_ = sp.tile([128, NCH], f32, tag="tc")
                        nc.scalar.activation(tc_[:], c_new[:], AF.Tanh)
                        h_new = stp.tile([128, NCH], f32, tag="h")
                        nc.vector.tensor_mul(h_new[:], sig[:, 2 * NCH:3 * NCH],
                                             tc_[:])
                        if t >= lw:
                            nc.vector.tensor_copy(
                                hsT[:, (t - lw):(t - lw) + llen * (NCH - 1) + 1:llen],
                                h_new[:])
                        h_cur, c_cur = h_new, c_new

                # ---------------- y + AllGather ----------------------------
                build_bcast(B1, dvfo_d, SH, pp)
                nc.scalar.activation(stage[:, :SH], hsT[:], AF.Relu)
                nc.vector.tensor_mul(stage[:, :SH], stage[:, :SH], B1[:, :SH])
                nc.sync.dma_start(yst_d.ap(), stage[:, :SH])
                if sim1 or fake_cc:
                    nc.sync.dma_start(yfull_d.ap()[0:128, :], yst_d.ap())
                else:
                    nc.gpsimd.collective_compute(
                        "AllGather", mybir.AluOpType.bypass,
                        ins=[yst_d.ap()], outs=[yfull_d.ap()],
                        replica_groups=rg)

                # ---------------- fin --------------------------------------
                nc.sync.dma_start(
                    tab[:, :8 * SH].rearrange("p (c t) -> p c t", c=8),
                    yfull_d.ap().rearrange("(c p) t -> p c t", p=128))
                nc.vector.memset(tab[:, NT0:NT0 + 1], 0.0)
                run_agg(("fin",), NT0 + 1, gp, ixp)
                build_bcast(B1, dv_d[("fin",)], SF, pp)
                # q = (Wo^T aggF) * dinv_t + bo   [64, SF]
                for t0 in range(0, SF, 512):
                    t1 = min(t0 + 512, SF)
                    ps = pp.tile([128, 512], f32, tag="mm", name=f"finmm{t0}")
                    nc.tensor.matmul(ps[:FOUT, :t1 - t0], Wo_sb[:],
                                     aggT[:, t0:t1], start=True, stop=True)
                    nc.vector.tensor_mul(ps[:FOUT, :t1 - t0], ps[:FOUT, :t1 - t0],
                                         B1[:FOUT, t0:t1])
                    nc.vector.tensor_scalar(stage[:FOUT, t0:t1],
                                            ps[:FOUT, :t1 - t0],
                                            bo_sb[:], None, ALU.add)
                # transpose to rows + log_softmax + store
                with tc.tile_pool(name="pH_s", bufs=4) as hp:
                    outb = const.tile([128, (SF // 128) * FOUT], f32, tag="outb")
                    for t in range(SF // 128):
                        pst = pp.tile([128, 512], f32, tag="mm", name=f"tp{t}")
                        nc.tensor.transpose(pst[:, :FOUT],
                                            stage[:FOUT, t * 128:(t + 1) * 128],
                                            ident_sb[:FOUT, :FOUT])
                        mx = hp.tile([128, 1], f32, tag="mx")
                        nc.vector.tensor_reduce(mx[:], pst[:, :FOUT],
                                                mybir.AxisListType.X, ALU.max)
                        sh = hp.tile([128, FOUT], f32, tag="sh")
                        nc.vector.tensor_scalar(sh[:], pst[:, :FOUT], mx[:],
                                                None, ALU.subtract)
                        ex = hp.tile([128, FOUT], f32, tag="ex")
                        se = hp.tile([128, 1], f32, tag="se")
                        nc.scalar.activation(ex[:], sh[:], AF.Exp,
                                             accum_out=se[:])
                        ln = hp.tile([128, 1], f32, tag="ln")
                        nc.scalar.activation(ln[:], se[:], AF.Ln)
                        nc.vector.tensor_scalar(
                            outb[:, t * FOUT:(t + 1) * FOUT],
                            sh[:], ln[:], None, ALU.subtract)
                    nc.sync.dma_start(
                        out_d.ap().rearrange("(t p) f -> p t f", p=128),
                        outb[:].rearrange("p (t f) -> p t f", f=FOUT))

    nc.compile()
    return nc


def kernel(**inputs):
    from concourse.bass_utils import run_bass_kernel_spmd
    in_maps, meta = preprocess(inputs)
    nc = build_program(meta)
    res = run_bass_kernel_spmd(nc, in_maps, list(range(meta["n_cores"])))
    SH = meta["SH"]
    full = np.zeros((N_NODES, FOUT), np.float32)
    for c in range(meta["n_cores"]):
        rows = res.results[c]["out"]
        full[c * SH:(c + 1) * SH] = rows[meta["fin_cols"][c]]
    return full



# revision 29
# speedup vs baseline: 1.0144x; 1.0144x over previous
"""Trainium2 Bass kernel for nn_DyHGCN (3-relation 2-layer GCN + LSTM + GCN head).

v2 architecture (8 NeuronCores, SPMD), gather-bound redesign:
  - All aggregations run as: feature-major table in SBUF [128 feats, nodes]
    -> gpsimd ap_gather (0.37 ns/idx, 15x faster than dma_gather) producing
    [128, slots] edge streams -> bucketed segmented tensor_reduce on DVE
    (segments = per-target in-edge lists padded to width%4, degree-sorted).
  - GCN algebra reordered as agg-then-W: agg_r uses tables pre-scaled by
    dinv_r[s] (host-side for x; on-device column scale for h1), the dinv[t]
    post-scale is applied as a PE-outer-product broadcast multiply. This
    kills the dense per-node W matmuls of v1 and all M matrices.
  - SPMD uniformity: per-(phase) segment-width histograms are maxed over
    cores (quota); shorter cores pad with dummy all-PADCOL segments, so one
    program fits all cores; only the index data differs.
  - Aggregation output columns are degree-sorted; later consumers absorb the
    permutation into their own index data; the final output is unpermuted on
    the host.
  - LSTM: 125 chunks x 20 steps per core, 32 warmup steps from zero state
    (host-validated rel err 5.5e-7), input taken directly from the
    feature-major osum tile (no DRAM round trip / transposes).
"""

import numpy as np

N_NODES = 20000
N_CORES = 8
F = 128
FOUT = 64
HALO = 64
LLEN = 20
LW = 32
CHUNK = 5632      # max gather slots per ap_gather call
NT0 = N_NODES     # l0 / fin table width (node-order)


# ---------------------------------------------------------------------------
# host-side preprocessing
# ---------------------------------------------------------------------------

def _csr_by_target(src, tgt, n):
    order = np.argsort(tgt, kind="stable")
    src_sorted = src[order]
    counts = np.bincount(tgt, minlength=n)
    starts = np.zeros(n + 1, np.int64)
    np.cumsum(counts, out=starts[1:])
    return src_sorted, starts


def _build_stream(widths_by_core, chunk_max=CHUNK, col_mult=None):
    """Uniform (SPMD) stream structure from per-core segment widths.

    Returns dict with:
      stream_w [nseg] widths (uniform), chunks: list of
        (slot_off, n_slots, col0, runs[(w, cnt)]), segtot, totslots,
      pos_by_width: {w: array of segment indices}.
    """
    # quota per width
    from collections import Counter
    quota = Counter()
    for wc in widths_by_core:
        c = Counter(wc.tolist())
        for w, n in c.items():
            quota[w] = max(quota[w], n)
    width_list = []
    for w in sorted(quota, reverse=True):
        width_list += [w] * quota[w]
    # chunk building (widths desc); pad each chunk tail to %16 with w4 dummies
    stream_w = []
    chunks = []
    cur_w = []
    cur_n = 0

    def close():
        nonlocal cur_w, cur_n
        pad = (16 - cur_n % 16) % 16
        while pad:
            cur_w.append(4)
            cur_n += 4
            pad -= 4
        runs = []
        for w in cur_w:
            if runs and runs[-1][0] == w:
                runs[-1][1] += 1
            else:
                runs.append([w, 1])
        chunks.append((len(stream_w), cur_n, [tuple(x) for x in runs]))
        stream_w.extend(cur_w)
        cur_w, cur_n = [], 0

    for w in width_list:
        if cur_n + w > chunk_max:
            close()
        cur_w.append(w)
        cur_n += w
    if cur_n:
        close()

    segtot = len(stream_w)
    if col_mult:
        m = (col_mult - segtot % col_mult) % col_mult
        if m:
            # m width-4 dummies; widen the last one so slot count is %16
            cur_w = [4] * m
            cur_n = 4 * m
            tail = (16 - cur_n % 16) % 16
            cur_w[-1] += tail
            cur_n += tail
            close()
            segtot = len(stream_w)
            assert segtot % col_mult == 0
    stream_w = np.asarray(stream_w, np.int64)
    seg_off = np.zeros(segtot + 1, np.int64)
    np.cumsum(stream_w, out=seg_off[1:])
    pos_by_width = {}
    for w in np.unique(stream_w):
        pos_by_width[int(w)] = np.nonzero(stream_w == w)[0]
    # final chunk descriptors with slot offsets and col0
    out_chunks = []
    for (seg0, n, runs) in chunks:
        out_chunks.append(dict(seg0=seg0, slot_off=int(seg_off[seg0]),
                               n=n, col0=seg0, runs=runs))
    return dict(stream_w=stream_w, seg_off=seg_off, chunks=out_chunks,
                segtot=segtot, totslots=int(seg_off[-1]),
                pos_by_width=pos_by_width)


def _assign_core(struct, widths_c, order_hint=None):
    """Assign this core's segments (widths_c) to stream positions.
    Returns seg_idx [ntgt]: stream segment index per target."""
    ntgt = len(widths_c)
    seg_idx = np.zeros(ntgt, np.int64)
    used = {w: 0 for w in struct["pos_by_width"]}
    # group targets by width, preserve target order within a width
    for w in np.unique(widths_c):
        tsel = np.nonzero(widths_c == w)[0]
        pos = struct["pos_by_width"][int(w)]
        k = used[int(w)]
        seg_idx[tsel] = pos[k:k + len(tsel)]
        used[int(w)] += len(tsel)
    return seg_idx


def _fill_idx(struct, seg_idx, tgt_nodes, deg, csr, colmap, padcol,
              self_col):
    """Build the int16 gather-index stream for one core/phase.

    tgt_nodes: node id per target (or -1 for dummy targets with deg 0).
    deg: in-degree (no self) per target. csr: (src_sorted, starts).
    colmap: maps node id -> table column. self_col: per-target self column
    (or -1 to skip self edge).
    """
    idx = np.full(struct["totslots"], padcol, np.int16)
    seg_off = struct["seg_off"]
    real = tgt_nodes >= 0
    rt = tgt_nodes[real]
    rdeg = deg[real]
    ss, st = csr
    starts = st[rt]
    total_e = int(rdeg.sum())
    if total_e:
        reps = np.repeat(np.arange(len(rt)), rdeg)
        intra = np.arange(total_e) - np.repeat(
            np.concatenate([[0], np.cumsum(rdeg)[:-1]]), rdeg)
        flat = starts[reps] + intra
        e_src = ss[flat]
        o = seg_off[seg_idx[real]]
        pos = np.repeat(o, rdeg) + intra
        idx[pos] = colmap[e_src].astype(np.int16)
    sc = self_col[real] if self_col is not None else None
    if sc is not None:
        idx[seg_off[seg_idx[real]] + rdeg] = sc.astype(np.int16)
    return idx


def _wrap_idx16(idx):
    n = len(idx)
    assert n % 16 == 0
    w = np.ascontiguousarray(idx.reshape(n // 16, 16).T)
    return np.tile(w, (8, 1))


def preprocess(inputs):
    SH = N_NODES // N_CORES
    x = np.asarray(inputs["x"], np.float32)
    srcs, tgts = [], []
    for r in range(3):
        ei = np.asarray(inputs[f"ei{r}"]).astype(np.int64)
        srcs.append(ei[0])
        tgts.append(ei[1])
    dinvs, csrs, degs = [], [], []
    for r in range(3):
        dg = np.bincount(tgts[r], minlength=N_NODES).astype(np.int64)
        degs.append(dg)
        dinvs.append((1.0 / np.sqrt(dg + 1.0)).astype(np.float32))
        csrs.append(_csr_by_target(srcs[r], tgts[r], N_NODES))
    all_src = np.concatenate(srcs)
    all_tgt = np.concatenate(tgts)
    deg_f = np.bincount(all_tgt, minlength=N_NODES).astype(np.int64)
    dinv_f = (1.0 / np.sqrt(deg_f + 1.0)).astype(np.float32)
    csr_f = _csr_by_target(all_src, all_tgt, N_NODES)

    def widthof(d):  # d = in-deg incl self
        return np.maximum(4, 4 * ((d + 3) // 4))

    own = [np.arange(c * SH, (c + 1) * SH, dtype=np.int64)
           for c in range(N_CORES)]
    l1tgt = []
    for c in range(N_CORES):
        halo = (np.arange(c * SH - HALO, c * SH, dtype=np.int64)
                if c > 0 else np.full(HALO, -1, np.int64))
        l1tgt.append(np.concatenate([halo, own[c]]))

    structs = {}
    for r in range(3):
        structs[("l0", r)] = _build_stream(
            [widthof(degs[r][own[c]] + 1) for c in range(N_CORES)])
    S0 = max(structs[("l0", r)]["segtot"] for r in range(3))
    # re-pad l0 streams to common segtot S0 (pad cols only matter for table
    # column addressing; easiest: use per-relation segtot, table stride = S0)
    for r in range(3):
        structs[("l1", r)] = _build_stream(
            [widthof(np.where(t >= 0, degs[r][np.maximum(t, 0)], 0) + 1)
             for c, t in enumerate(l1tgt)])
    S1 = max(structs[("l1", r)]["segtot"] for r in range(3))
    structs[("fin",)] = _build_stream(
        [widthof(deg_f[own[c]] + 1) for c in range(N_CORES)], col_mult=128)
    SF = structs[("fin",)]["segtot"]

    PAD0 = NT0            # l0/fin table pad column
    PAD1 = 8 * S0         # l1 table pad column
    NTAB1 = 8 * S0 + 1
    assert NTAB1 <= 32768 and S1 + 1 <= 32768 and SF <= 32767

    meta = dict(SH=SH, S0=S0, S1=S1, SF=SF, NTAB1=NTAB1,
                halo=HALO, llen=LLEN, lw=LW, n_cores=N_CORES,
                structs=structs)

    # per-core, per-relation l0 seg assignment (needed globally for l1 cols)
    segidx_l0 = {}   # (r, c) -> seg per own target
    colmap_l1 = {}   # r -> [N] table column in h1_r table
    for r in range(3):
        cm = np.zeros(N_NODES, np.int64)
        for c in range(N_CORES):
            w = widthof(degs[r][own[c]] + 1)
            si = _assign_core(structs[("l0", r)], w)
            segidx_l0[(r, c)] = si
            cm[own[c]] = c * S0 + si
        colmap_l1[r] = cm

    ident_colmap = np.arange(N_NODES, dtype=np.int64)

    shared = {
        "WihT": None, "WhhT": None,  # filled below
    }
    WihT = np.asarray(inputs["lstm_Wih"], np.float32)
    WhhT = np.asarray(inputs["lstm_Whh"], np.float32)
    perm = np.concatenate([np.arange(0, 128), np.arange(128, 256),
                           np.arange(384, 512), np.arange(256, 384)])
    lstm_b = (np.asarray(inputs["lstm_bih"], np.float32)
              + np.asarray(inputs["lstm_bhh"], np.float32))[perm]
    meta["lstm_bias_nonzero"] = bool(np.any(lstm_b != 0.0))
    import ml_dtypes
    bf16 = ml_dtypes.bfloat16
    gcn_bz = all(not np.any(np.asarray(inputs[f"b_r{r}_l{l}"]))
                 for r in range(3) for l in range(2))
    bo_z = not np.any(np.asarray(inputs["bo"]))
    meta["gcn_bias_zero"] = gcn_bz
    meta["bo_zero"] = bo_z
    shared = {
        "WihT": np.ascontiguousarray(WihT[perm].T).astype(bf16),
        "WhhT": np.ascontiguousarray(WhhT[perm].T).astype(bf16),
        "lstm_b": np.ascontiguousarray(lstm_b.reshape(4, 128).T),
        "Wo": np.asarray(inputs["Wo"], np.float32).astype(bf16),
        "bo": np.asarray(inputs["bo"], np.float32).reshape(FOUT, 1),
        "ones1": np.ones((1, 128), np.float32),
        "ident": np.eye(128, dtype=np.float32),
    }
    for r in range(3):
        for l in range(2):
            shared[f"W_r{r}_l{l}"] = np.asarray(
                inputs[f"W_r{r}_l{l}"], np.float32).astype(bf16)
            shared[f"b_r{r}_l{l}"] = np.asarray(
                inputs[f"b_r{r}_l{l}"], np.float32).reshape(128, 1)
        # pre-scaled transposed x tables (zero pad col handled on device)
        shared[f"xt{r}"] = np.ascontiguousarray(
            (x * dinvs[r][:, None]).T)

    in_maps = []
    fin_cols = []    # per core: column of each own target in fin stream
    for c in range(N_CORES):
        m = dict(shared)
        for r in range(3):
            st0 = structs[("l0", r)]
            si0 = segidx_l0[(r, c)]
            m[f"l0r{r}_idx"] = _wrap_idx16(_fill_idx(
                st0, si0, own[c], degs[r][own[c]], csrs[r],
                ident_colmap, PAD0, own[c]))
            dv = np.zeros(st0["segtot"], np.float32)
            dv[si0] = dinvs[r][own[c]] ** 2 if gcn_bz else dinvs[r][own[c]]
            m[f"l0r{r}_dv"] = dv.reshape(1, -1)

            st1 = structs[("l1", r)]
            t1 = l1tgt[c]
            d1 = np.where(t1 >= 0, degs[r][np.maximum(t1, 0)], 0)
            w1 = widthof(np.where(t1 >= 0, d1 + 1, 0))
            si1 = _assign_core(st1, w1)
            selfc = np.where(t1 >= 0, colmap_l1[r][np.maximum(t1, 0)], 0)
            m[f"l1r{r}_idx"] = _wrap_idx16(_fill_idx(
                st1, si1, t1, d1, csrs[r], colmap_l1[r], PAD1, selfc))
            dv1 = np.zeros(st1["segtot"], np.float32)
            rl = t1 >= 0
            dv1[si1[rl]] = dinvs[r][t1[rl]]
            m[f"l1r{r}_dv"] = dv1.reshape(1, -1)
            # osum reorder: node-order j -> l1 column (core0 halo -> zerocol)
            reo = np.full(2576, S1, np.int16)
            cols = si1.astype(np.int64)
            if c == 0:
                reo[:HALO] = S1
                reo[HALO:HALO + SH] = cols[HALO:]
            else:
                reo[:HALO + SH] = cols
            m[f"reo{r}"] = _wrap_idx16(reo)

        stf = structs[("fin",)]
        wf = widthof(deg_f[own[c]] + 1)
        sif = _assign_core(stf, wf)
        fin_cols.append(sif)
        m["fin_idx"] = _wrap_idx16(_fill_idx(
            stf, sif, own[c], deg_f[own[c]], csr_f,
            ident_colmap, PAD0, own[c]))
        dvf = np.zeros(stf["segtot"], np.float32)
        dvf[sif] = dinv_f[own[c]]
        m["fin_dv"] = dvf.reshape(1, -1)
        m["dvf_own"] = dinv_f[own[c]].reshape(1, -1).astype(np.float32)
        in_maps.append(m)

    meta["fin_cols"] = fin_cols
    return in_maps, meta


# ---------------------------------------------------------------------------
# bass program
# ---------------------------------------------------------------------------

def build_program(meta, sim1=False, fake_cc=False):
    import concourse.bass as bass
    import concourse.tile as tile
    from concourse import bacc, mybir, library_config

    f32 = mybir.dt.float32
    bf16 = mybir.dt.bfloat16
    i16 = mybir.dt.int16
    AF = mybir.ActivationFunctionType
    ALU = mybir.AluOpType
    zb = meta["gcn_bias_zero"]
    bo_z = meta["bo_zero"]

    SH = meta["SH"]
    S0, S1, SF = meta["S0"], meta["S1"], meta["SF"]
    NTAB1 = meta["NTAB1"]
    NTABMAX = max(NT0 + 1, NTAB1)
    halo, llen, lw = meta["halo"], meta["llen"], meta["lw"]
    NCH = SH // llen
    n_cores = meta["n_cores"]
    structs = meta["structs"]
    lstm_bias = meta["lstm_bias_nonzero"]
    SMAX = max(S0, S1 + 1, SF)
    rg = [list(range(n_cores))]

    nc = bacc.Bacc("TRN2", target_bir_lowering=False, debug=False,
                   num_devices=1 if sim1 else n_cores, num_swdge_queues=4)

    def inp(name, shape, dtype=f32):
        return nc.dram_tensor(name, list(shape), dtype, kind="ExternalInput")

    xt_d = [inp(f"xt{r}", (128, NT0)) for r in range(3)]
    Wz_d = {(r, l): inp(f"W_r{r}_l{l}", (128, 128), bf16)
            for r in range(3) for l in range(2)}
    bz_d = {(r, l): inp(f"b_r{r}_l{l}", (128, 1))
            for r in range(3) for l in range(2)}
    WihT_d = inp("WihT", (128, 512), bf16)
    WhhT_d = inp("WhhT", (128, 512), bf16)
    lstmb_d = inp("lstm_b", (128, 4))
    Wo_d = inp("Wo", (128, FOUT), bf16)
    bo_d = inp("bo", (FOUT, 1))
    ones1_d = inp("ones1", (1, 128))
    ident_d = inp("ident", (128, 128))
    idx_d, dv_d, reo_d = {}, {}, {}
    for r in range(3):
        idx_d[("l0", r)] = inp(f"l0r{r}_idx",
                               (128, structs[("l0", r)]["totslots"] // 16), i16)
        dv_d[("l0", r)] = inp(f"l0r{r}_dv", (1, structs[("l0", r)]["segtot"]))
        idx_d[("l1", r)] = inp(f"l1r{r}_idx",
                               (128, structs[("l1", r)]["totslots"] // 16), i16)
        dv_d[("l1", r)] = inp(f"l1r{r}_dv", (1, structs[("l1", r)]["segtot"]))
        reo_d[r] = inp(f"reo{r}", (128, 2576 // 16), i16)
    idx_d[("fin",)] = inp("fin_idx", (128, structs[("fin",)]["totslots"] // 16), i16)
    dv_d[("fin",)] = inp("fin_dv", (1, SF))
    dvfo_d = inp("dvf_own", (1, SH))

    out_d = nc.dram_tensor("out", [SF, FOUT], f32, kind="ExternalOutput")

    h1st_d = [nc.dram_tensor(f"h1st{r}", [128, S0], f32) for r in range(3)]
    h1full_d = [nc.dram_tensor(f"h1full{r}", [n_cores * 128, S0], f32,
                               addr_space="Shared") for r in range(3)]
    yst_d = nc.dram_tensor("yst", [128, SH], f32)
    yfull_d = nc.dram_tensor("yfull", [n_cores * 128, SH], f32,
                             addr_space="Shared")

    with tile.TileContext(nc) as tc:
        nc.gpsimd.load_library(library_config.ap_gather)
        import contextlib
        st = contextlib.ExitStack()
        with st:
            const = st.enter_context(tc.tile_pool(name="const", bufs=1))

            def load_const(dram, shape, dtype=f32):
                t = const.tile(list(shape), dtype, tag=dram.name,
                               name=f"c_{dram.name}")
                nc.sync.dma_start(t[:], dram.ap())
                return t

            Wz_sb = {k: load_const(v, (128, 128), bf16) for k, v in Wz_d.items()}
            bz_sb = {k: load_const(v, (128, 1)) for k, v in bz_d.items()}
            ident_sb = load_const(ident_d, (128, 128))
            ones1_sb = load_const(ones1_d, (1, 128))
            Wo_sb = load_const(Wo_d, (128, FOUT), bf16)
            bo_sb = load_const(bo_d, (FOUT, 1))

            tab = const.tile([128, NTABMAX], f32, tag="tab")
            aggT16 = const.tile([128, SMAX], bf16, tag="aggT16")
            stage = const.tile([128, SMAX], f32, tag="stage")
            osum = const.tile([128, 2576], f32, tag="osum")
            hsT = const.tile([128, SH], f32, tag="hsT")

            # ---------------- helpers --------------------------------------
            def run_agg(ph_key, ntab, gp, ixp):
                """ap_gather + segmented reduces into aggT[:, :segtot]."""
                stt = structs[ph_key]
                idxd = idx_d[ph_key]
                for ci, ch in enumerate(stt["chunks"]):
                    n = ch["n"]
                    o16 = ch["slot_off"] // 16
                    ix = ixp.tile([128, CHUNK // 16], i16, tag=f"ix{ci % 2}",
                                  name=f"ix_{ph_key}_{ci}")
                    nc.sync.dma_start(ix[:, :n // 16],
                                      idxd.ap()[:, o16:o16 + n // 16])
                    g = gp.tile([128, CHUNK], f32, tag=f"g{ci % 2}",
                                name=f"g_{ph_key}_{ci}")
                    nc.gpsimd.ap_gather(
                        g[:, :n], tab[:, :ntab], ix[:, :n // 16],
                        channels=128, num_elems=ntab, d=1, num_idxs=n)
                    col = ch["col0"]
                    off = 0
                    for (w, cnt) in ch["runs"]:
                        with nc.allow_low_precision(
                                reason="bf16 seg-sum, tol 2e-2"):
                            nc.vector.tensor_reduce(
                                aggT16[:, col:col + cnt],
                                g[:, off:off + cnt * w].rearrange(
                                    "p (s k) -> p s k", k=w),
                                mybir.AxisListType.X, ALU.add)
                        col += cnt
                        off += cnt * w

            def build_bcast(row_d, width, pp, nparts=128):
                """returns B1[:nparts, :width] = ones^T @ row (broadcast)."""
                dst = bp.tile([128, SMAX], f32, tag="B1",
                              name=f"B1_{row_d.name}")
                row = const.tile([1, SMAX], f32, tag="dvrow")
                nc.sync.dma_start(row[:, :width], row_d.ap())
                for t0 in range(0, width, 512):
                    t1 = min(t0 + 512, width)
                    ps = pp.tile([128, 512], f32, tag="bc",
                                 name=f"bc{id(row_d)}_{t0}")
                    nc.tensor.matmul(ps[:nparts, :t1 - t0],
                                     ones1_sb[:, :nparts],
                                     row[:, t0:t1], start=True, stop=True)
                    nc.vector.tensor_copy(dst[:nparts, t0:t1],
                                          ps[:nparts, :t1 - t0])
                return dst

            # ---------------- l0 + l1 over relations ----------------------
            with tc.tile_pool(name="gp", bufs=1) as gp, \
                 tc.tile_pool(name="ixp", bufs=1) as ixp, \
                 tc.tile_pool(name="bp", bufs=1) as bp, \
                 tc.tile_pool(name="pp", bufs=2, space="PSUM") as pp:

                nc.vector.memset(osum[:], 0.0)

                for r in range(3):
                    # table: pre-scaled x (node order) + zero pad col
                    nc.sync.dma_start(tab[:, :NT0], xt_d[r].ap())
                    nc.vector.memset(tab[:, NT0:NT0 + 1], 0.0)
                    run_agg(("l0", r), NT0 + 1, gp, ixp)
                    s0r = structs[("l0", r)]["segtot"]
                    B1 = build_bcast(dv_d[("l0", r)], s0r, pp)
                    # h1 = relu((W^T agg) * dinv_t + b) * dinv_t
                    for t0 in range(0, s0r, 512):
                        t1 = min(t0 + 512, s0r)
                        ps = pp.tile([128, 512], f32, tag="mm",
                                     name=f"l0mm{r}_{t0}")
                        nc.tensor.matmul(ps[:, :t1 - t0], Wz_sb[(r, 0)][:],
                                         aggT16[:, t0:t1], start=True, stop=True)
                        if zb:
                            # b==0: relu(z*d)*d == relu(z)*d^2 (B1 holds d^2)
                            nc.scalar.activation(stage[:, t0:t1],
                                                 ps[:, :t1 - t0], AF.Relu)
                            nc.vector.tensor_mul(stage[:, t0:t1],
                                                 stage[:, t0:t1], B1[:, t0:t1])
                        else:
                            nc.vector.tensor_mul(ps[:, :t1 - t0],
                                                 ps[:, :t1 - t0], B1[:, t0:t1])
                            nc.scalar.activation(stage[:, t0:t1], ps[:, :t1 - t0],
                                                 AF.Relu, bias=bz_sb[(r, 0)][:])
                            nc.vector.tensor_mul(stage[:, t0:t1], stage[:, t0:t1],
                                                 B1[:, t0:t1])
                    if s0r < S0:
                        nc.vector.memset(stage[:, s0r:S0], 0.0)
                    nc.sync.dma_start(h1st_d[r].ap(), stage[:, :S0])
                    if sim1 or fake_cc:
                        nc.sync.dma_start(h1full_d[r].ap()[0:128, :],
                                          h1st_d[r].ap())
                    else:
                        nc.gpsimd.collective_compute(
                            "AllGather", mybir.AluOpType.bypass,
                            ins=[h1st_d[r].ap()], outs=[h1full_d[r].ap()],
                            replica_groups=rg)

                for r in range(3):
                    # l1 table: AllGathered h1 (8 shards side by side) + pad
                    nc.sync.dma_start(
                        tab[:, :8 * S0].rearrange("p (c t) -> p c t", c=8),
                        h1full_d[r].ap().rearrange("(c p) t -> p c t", p=128))
                    nc.vector.memset(tab[:, 8 * S0:8 * S0 + 1], 0.0)
                    run_agg(("l1", r), NTAB1, gp, ixp)
                    s1r = structs[("l1", r)]["segtot"]
                    B1 = build_bcast(dv_d[("l1", r)], s1r, pp)
                    for t0 in range(0, s1r, 512):
                        t1 = min(t0 + 512, s1r)
                        ps = pp.tile([128, 512], f32, tag="mm",
                                     name=f"l1mm{r}_{t0}")
                        nc.tensor.matmul(ps[:, :t1 - t0], Wz_sb[(r, 1)][:],
                                         aggT16[:, t0:t1], start=True, stop=True)
                        nc.vector.tensor_mul(ps[:, :t1 - t0], ps[:, :t1 - t0],
                                             B1[:, t0:t1])
                        nc.scalar.activation(stage[:, t0:t1], ps[:, :t1 - t0],
                                             AF.Relu, bias=bz_sb[(r, 1)][:])
                    nc.vector.memset(stage[:, S1:S1 + 1], 0.0)
                    # reorder columns to node order and accumulate into osum
                    reo_sb = ixp.tile([128, 2576 // 16], i16, tag="reo",
                                      name=f"reo{r}")
                    nc.sync.dma_start(reo_sb[:], reo_d[r].ap())
                    g = gp.tile([128, CHUNK], f32, tag="g0", name=f"reog{r}")
                    nc.gpsimd.ap_gather(
                        g[:, :2576], stage[:, :S1 + 1], reo_sb[:],
                        channels=128, num_elems=S1 + 1, d=1, num_idxs=2576)
                    nc.vector.tensor_add(osum[:], osum[:], g[:, :2576])

                # ---------------- LSTM --------------------------------------
                osum16 = bp.tile([128, 2576], bf16, tag="B1",
                                 name="osum16")
                nc.scalar.activation(osum16[:], osum[:], AF.Copy)
                with tc.tile_pool(name="pE_ps", bufs=2, space="PSUM") as ppe, \
                     tc.tile_pool(name="pE_w", bufs=1) as wp, \
                     tc.tile_pool(name="pE_s", bufs=1) as sp, \
                     tc.tile_pool(name="pE_st", bufs=2) as stp:
                    WihT_sb = wp.tile([128, 512], bf16, tag="wih")
                    nc.sync.dma_start(WihT_sb[:], WihT_d.ap())
                    WhhT_sb = wp.tile([128, 512], bf16, tag="whh")
                    nc.sync.dma_start(WhhT_sb[:], WhhT_d.ap())
                    if lstm_bias:
                        lb_sb = wp.tile([128, 4], f32, tag="lb")
                        nc.sync.dma_start(lb_sb[:], lstmb_d.ap())
                    h_cur = stp.tile([128, NCH], bf16, tag="h")
                    c_cur = stp.tile([128, NCH], f32, tag="c")
                    nc.vector.memset(h_cur[:], 0.0)
                    nc.vector.memset(c_cur[:], 0.0)
                    for t in range(lw + llen):
                        ps = ppe.tile([128, 4 * NCH], f32, tag="gates")
                        xsl = osum16[:, (halo - lw) + t:
                                     (halo - lw) + t + llen * (NCH - 1) + 1:llen]
                        for gi in range(4):
                            nc.tensor.matmul(ps[:, gi * NCH:(gi + 1) * NCH],
                                             WihT_sb[:, gi * 128:(gi + 1) * 128],
                                             xsl, start=True, stop=False)
                            nc.tensor.matmul(ps[:, gi * NCH:(gi + 1) * NCH],
                                             WhhT_sb[:, gi * 128:(gi + 1) * 128],
                                             h_cur[:], start=False, stop=True)
                        sig = sp.tile([128, 3 * NCH], f32, tag="sig")
                        gg = sp.tile([128, NCH], f32, tag="gg")
                        if lstm_bias:
                            for k in range(3):
                                nc.scalar.activation(
                                    sig[:, k * NCH:(k + 1) * NCH],
                                    ps[:, k * NCH:(k + 1) * NCH],
                                    AF.Sigmoid, bias=lb_sb[:, k:k + 1])
                            nc.scalar.activation(gg[:], ps[:, 3 * NCH:4 * NCH],
                                                 AF.Tanh, bias=lb_sb[:, 3:4])
                        else:
                            nc.scalar.activation(sig[:], ps[:, 0:3 * NCH],
                                                 AF.Sigmoid)
                            nc.scalar.activation(gg[:], ps[:, 3 * NCH:4 * NCH],
                                                 AF.Tanh)
                        fc = sp.tile([128, NCH], f32, tag="fc")
                        nc.vector.tensor_mul(fc[:], sig[:, NCH:2 * NCH], c_cur[:])
                        ig = sp.tile([128, NCH], f32, tag="ig")
                        nc.vector.tensor_mul(ig[:], sig[:, 0:NCH], gg[:])
                        c_new = stp.tile([128, NCH], f32, tag="c")
                        nc.vector.tensor_add(c_new[:], fc[:], ig[:])
                        tc_ = sp.tile([128, NCH], f32, tag="tc")
                        nc.scalar.activation(tc_[:], c_new[:], AF.Tanh)
                        h_new = stp.tile([128, NCH], bf16, tag="h")
                        nc.vector.tensor_mul(h_new[:], sig[:, 2 * NCH:3 * NCH],
                                             tc_[:])
                        if t >= lw:
                            nc.vector.tensor_copy(
                                hsT[:, (t - lw):(t - lw) + llen * (NCH - 1) + 1:llen],
                                h_new[:])
                        h_cur, c_cur = h_new, c_new

                # ---------------- y + AllGather ----------------------------
                B1 = build_bcast(dvfo_d, SH, pp)
                nc.scalar.activation(stage[:, :SH], hsT[:], AF.Relu)
                nc.vector.tensor_mul(stage[:, :SH], stage[:, :SH], B1[:, :SH])
                nc.sync.dma_start(yst_d.ap(), stage[:, :SH])
                if sim1 or fake_cc:
                    nc.sync.dma_start(yfull_d.ap()[0:128, :], yst_d.ap())
                else:
                    nc.gpsimd.collective_compute(
                        "AllGather", mybir.AluOpType.bypass,
                        ins=[yst_d.ap()], outs=[yfull_d.ap()],
                        replica_groups=rg)

                # ---------------- fin --------------------------------------
                nc.sync.dma_start(
                    tab[:, :8 * SH].rearrange("p (c t) -> p c t", c=8),
                    yfull_d.ap().rearrange("(c p) t -> p c t", p=128))
                nc.vector.memset(tab[:, NT0:NT0 + 1], 0.0)
                run_agg(("fin",), NT0 + 1, gp, ixp)
                B1 = build_bcast(dv_d[("fin",)], SF, pp)
                # q = (Wo^T aggF) * dinv_t + bo   [64, SF]
                for t0 in range(0, SF, 512):
                    t1 = min(t0 + 512, SF)
                    ps = pp.tile([128, 512], f32, tag="mm", name=f"finmm{t0}")
                    nc.tensor.matmul(ps[:FOUT, :t1 - t0], Wo_sb[:],
                                     aggT16[:, t0:t1], start=True, stop=True)
                    if bo_z:
                        nc.vector.tensor_mul(stage[:FOUT, t0:t1],
                                             ps[:FOUT, :t1 - t0],
                                             B1[:FOUT, t0:t1])
                    else:
                        nc.vector.tensor_mul(ps[:FOUT, :t1 - t0],
                                             ps[:FOUT, :t1 - t0],
                                             B1[:FOUT, t0:t1])
                        nc.vector.tensor_scalar(stage[:FOUT, t0:t1],
                                                ps[:FOUT, :t1 - t0],
                                                bo_sb[:], None, ALU.add)
                # transpose to rows + log_softmax + store
                with tc.tile_pool(name="pH_s", bufs=4) as hp:
                    outb = const.tile([128, (SF // 128) * FOUT], f32, tag="outb")
                    for t in range(SF // 128):
                        pst = pp.tile([128, 512], f32, tag="mm", name=f"tp{t}")
                        nc.tensor.transpose(pst[:, :FOUT],
                                            stage[:FOUT, t * 128:(t + 1) * 128],
                                            ident_sb[:FOUT, :FOUT])
                        mx = hp.tile([128, 1], f32, tag="mx")
                        nc.vector.tensor_reduce(mx[:], pst[:, :FOUT],
                                                mybir.AxisListType.X, ALU.max)
                        sh = hp.tile([128, FOUT], f32, tag="sh")
                        nc.vector.tensor_scalar(sh[:], pst[:, :FOUT], mx[:],
                                                None, ALU.subtract)
                        ex = hp.tile([128, FOUT], f32, tag="ex")
                        se = hp.tile([128, 1], f32, tag="se")
                        nc.scalar.activation(ex[:], sh[:], AF.Exp,
                                             accum_out=se[:])
                        ln = hp.tile([128, 1], f32, tag="ln")
                        nc.scalar.activation(ln[:], se[:], AF.Ln)
                        nc.vector.tensor_scalar(
                            outb[:, t * FOUT:(t + 1) * FOUT],
                            sh[:], ln[:], None, ALU.subtract)
                    nc.sync.dma_start(
                        out_d.ap().rearrange("(t p) f -> p t f", p=128),
                        outb[:].rearrange("p (t f) -> p t f", f=FOUT))

    nc.compile()
    return nc


def kernel(**inputs):
    from concourse.bass_utils import run_bass_kernel_spmd
    in_maps, meta = preprocess(inputs)
    nc = build_program(meta)
    res = run_bass_kernel_spmd(nc, in_maps, list(range(meta["n_cores"])))
    SH = meta["SH"]
    full = np.zeros((N_NODES, FOUT), np.float32)
    for c in range(meta["n_cores"]):
        rows = res.results[c]["out"]
        full[c * SH:(c + 1) * SH] = rows[meta["fin_cols"][c]]
    return full

